# revision 1
# baseline (speedup 1.0000x reference)
"""Trainium2 Bass kernel for the NodeAttentionLayer (GAT-style) problem.

Math (per reference.py):
    h_t = t_input @ W_t; h_o = o_input @ W_o
    s_t = h_t @ a[:F];  s_o = h_o @ a[F:]
    e[i,j]   = leaky_relu(s_t[i] + s_o[j], 0.2)
    att      = softmax(where(adj>0, e, -9e15), axis=1)
    out      = elu(att @ h_o)

Key identity used on-device: with y = s_t[i] + s_o[j], c = (y > 0),
    exp(lrelu(y)) = c * u1[i] v1[j] + (1-c) * u2[i] v2[j]
where u1 = exp(s_t), v1 = exp(s_o), u2 = exp(0.2 s_t), v2 = exp(0.2 s_o).
So with M1 = adj * c and M2 = adj - M1 (both 0/1 masks):
    att-numer @ h_o = u1 * (v1*h_o_ext).T @ M1 + u2 * (v2*h_o_ext).T @ M2
(h_o_ext = [h_o | 1] supplies the softmax denominator as row F). Softmax and
the exp of the max-trick cancel in the ratio, and dividing numerator and
denominator by u2 leaves a single per-column factor r = exp(0.8 s_t).

Sharding: rows of t_input/adj (N_t) split across 8 cores; o_input replicated.
The kernel computes output TRANSPOSED ([F, rows]) per core; host transposes
back.  adj is fed per-core as adj[rows,:].T in bf16 (0/1 -> lossless).
"""

import contextlib
import ctypes
import sys
import tempfile
import types

import ml_dtypes
import numpy as np

import concourse.bass as bass
import concourse.mybir as mybir
import concourse.tile as tile
from concourse.vector_clock import ScopedClock

bf16 = ml_dtypes.bfloat16

# ---------------------------------------------------------------------------
# Environment shims
# ---------------------------------------------------------------------------

def _patch_tile_drain():
    """walrus in this container allows only one sync-wait per sync-engine
    instruction; split the TileContext epilogue drain's waits onto
    individual nops."""
    if getattr(tile.TileContext, "_drain_patch_installed", False):
        return

    def _drain_and_barrier(self, tick_clock, wait_clock):
        nop_inst = self.nc.sync.nop(nofuse=True)
        wait_clock.add_sem_waits(
            nop_inst.ins, ScopedClock({None: tick_clock.global_clock})
        )
        ow = list(nop_inst.ins.sync_info.on_wait) if nop_inst.ins.sync_info else []
        if len(ow) > 1:
            nop_inst.ins.sync_info.on_wait = ow[:1]
            for w in ow[1:]:
                extra = self.nc.sync.nop(nofuse=True)
                if extra.ins.sync_info is None:
                    extra.ins.sync_info = mybir.SyncInfo(on_wait=[w], on_update=[])
                else:
                    extra.ins.sync_info.on_wait = [w]
        self.nc.sync.drain()
        self.nc.all_engine_barrier()
        popped = self.nc._tile_sem_poison_stack.pop()
        assert popped is self._sem_poison
        self.nc.clear_and_free_semaphores(list(self.sems.allocated().values()))
        self.nc.all_engine_barrier()

    tile.TileContext._drain_and_barrier = _drain_and_barrier
    tile.TileContext._drain_patch_installed = True


def _install_ntff_hook():
    """Provide antenv.axon_hooks (absent in this image) so trace=True works."""
    if "antenv.axon_hooks" in sys.modules:
        return
    import antenv

    state = {"hook": None}
    mod = types.ModuleType("antenv.axon_hooks")
    mod.set_axon_ntff_profile_hook = lambda h: state.__setitem__("hook", h)
    mod.get_axon_ntff_profile_hook = lambda: state["hook"]
    sys.modules["antenv.axon_hooks"] = mod
    antenv.axon_hooks = mod

    try:
        lib = ctypes.CDLL("/opt/axon/libaxon_pjrt.so")
    except OSError:
        return
    if not hasattr(lib, "axon_start_nrt_profile"):
        return
    lib.axon_start_nrt_profile.argtypes = [
        ctypes.POINTER(ctypes.c_int64),
        ctypes.c_size_t,
    ]
    lib.axon_start_nrt_profile.restype = ctypes.c_int64
    lib.axon_stop_nrt_profile.argtypes = [ctypes.c_char_p]
    lib.axon_stop_nrt_profile.restype = ctypes.c_int64

    @contextlib.contextmanager
    def _ntff_hook(output_dir, device_ids):
        import jax

        jax.devices()
        if device_ids:
            ids = (ctypes.c_int64 * len(device_ids))(*device_ids)
            rc = lib.axon_start_nrt_profile(ids, len(device_ids))
        else:
            rc = lib.axon_start_nrt_profile(None, 0)
        if rc != 0:
            raise RuntimeError(f"axon_start_nrt_profile rc={rc}")
        try:
            yield
        finally:
            n = lib.axon_stop_nrt_profile(str(output_dir).encode())
            print(f"profile: {n} file(s) written to {output_dir}", file=sys.stderr)

    state["hook"] = _ntff_hook


_patch_tile_drain()
_install_ntff_hook()


def _split_multi_waits(nc):
    """walrus here accepts at most ONE sync-wait per instruction; hoist extra
    waits onto same-engine nops inserted immediately before."""
    import bass_rust

    k = 0
    for f in nc.m.functions:
        for blk in f.blocks:
            insts = blk.instructions
            out = []
            changed = False
            for inst in insts:
                si = inst.sync_info
                ow = list(si.on_wait) if si is not None else []
                if len(ow) > 1:
                    for w in ow[:-1]:
                        nop = bass_rust.InstNoOp(
                            name=f"waitsplit-{k}", engine=inst.engine
                        )
                        k += 1
                        nop.sync_info = mybir.SyncInfo(on_wait=[w], on_update=[])
                        out.append(nop)
                    si.on_wait = [ow[-1]]
                    changed = True
                out.append(inst)
            if changed:
                blk.instructions = out

# ---------------------------------------------------------------------------
# Problem constants (hardcoded per spec)
# ---------------------------------------------------------------------------
N_T, N_O, F_IN, F_OUT = 8192, 8192, 256, 64
N_CORES = 8
R = N_T // N_CORES            # rows (i) per core = 1024
NJ = N_O // 128               # j tiles of 128 = 64
KC = F_IN // 128              # contraction chunks for projections = 2
GROUP = 16                    # j-tiles per setup group
NG = NJ // GROUP              # setup groups = 4
ALPHA = 0.2
F32 = mybir.dt.float32
BF16 = mybir.dt.bfloat16
AF = mybir.ActivationFunctionType
OP = mybir.AluOpType


def _ap_bcast_partitions(ap, n):
    """AP view replicating a [1, ...] access pattern across n partitions."""
    return bass.AP(tensor=ap.tensor, offset=ap.offset, ap=[[0, n]] + list(ap.ap[1:]))


def _ap_repeat_free(ap, reps):
    """AP view of a [P, K] tile as [P, K, reps] (innermost step-0 repeat)."""
    return bass.AP(
        tensor=ap.tensor, offset=ap.offset, ap=list(ap.ap) + [[0, reps]]
    )


def build_kernel(split_waits=True):
    nc = bass.Bass("TRN2")

    t_T = nc.dram_tensor("t_T", [F_IN, R], F32, kind="ExternalInput")
    o_T = nc.dram_tensor("o_T", [F_IN, N_O], F32, kind="ExternalInput")
    w_t = nc.dram_tensor("w_t", [F_IN, F_OUT], F32, kind="ExternalInput")
    w_o = nc.dram_tensor("w_o", [F_IN, F_OUT], F32, kind="ExternalInput")
    a_vec = nc.dram_tensor("a_vec", [2 * F_OUT, 1], F32, kind="ExternalInput")
    adjT = nc.dram_tensor("adjT", [N_O, R], BF16, kind="ExternalInput")
    out = nc.dram_tensor("out", [F_OUT, R], F32, kind="ExternalOutput")

    with tile.TileContext(nc) as tc, contextlib.ExitStack() as ctx:
        singles = ctx.enter_context(tc.tile_pool(name="singles", bufs=1))
        stage = ctx.enter_context(tc.tile_pool(name="stage", bufs=2))
        adj_pool = ctx.enter_context(tc.tile_pool(name="adj", bufs=6))
        c_pool = ctx.enter_context(tc.tile_pool(name="cmask", bufs=4))
        m1_pool = ctx.enter_context(tc.tile_pool(name="m1", bufs=4))
        m2_pool = ctx.enter_context(tc.tile_pool(name="m2", bufs=4))
        acc_psum = ctx.enter_context(tc.tile_pool(name="acc", bufs=1, space="PSUM"))
        misc_psum = ctx.enter_context(tc.tile_pool(name="mpsum", bufs=2, space="PSUM"))

        # ------------------------------------------------------------------
        # Setup: weights + t-side scalars
        # ------------------------------------------------------------------
        wt_sb = singles.tile([128, KC, F_OUT], F32)
        wo_sb = singles.tile([128, KC, F_OUT], F32)
        for c in range(KC):
            nc.sync.dma_start(out=wt_sb[:, c, :], in_=w_t[c * 128:(c + 1) * 128, :])
            nc.sync.dma_start(out=wo_sb[:, c, :], in_=w_o[c * 128:(c + 1) * 128, :])
        a_t_sb = singles.tile([F_OUT, 1], F32)
        nc.sync.dma_start(out=a_t_sb[:, :], in_=a_vec[0:F_OUT, :])
        # a_o broadcast to [128, F_OUT]
        a_o_b = singles.tile([128, F_OUT], F32)
        nc.sync.dma_start(
            out=a_o_b[:, :],
            in_=bass.AP(tensor=a_vec, offset=F_OUT, ap=[[0, 128], [1, F_OUT]]),
        )
        ones_row = singles.tile([1, F_OUT + 1], F32)
        nc.vector.memset(ones_row[:, :], 1.0)

        t_T_sb = singles.tile([128, KC, R], F32)
        for c in range(KC):
            nc.sync.dma_start(out=t_T_sb[:, c, :], in_=t_T[c * 128:(c + 1) * 128, :])

        # h_tT [F_OUT, R] = W_t.T @ t_blk.T   (psum, 2 k-chunks x 2 n-chunks)
        ht_sb = singles.tile([F_OUT, R], F32)
        for n in range(R // 512):
            ht_ps = misc_psum.tile([F_OUT, 512], F32, tag="mps")
            for c in range(KC):
                nc.tensor.matmul(
                    ht_ps[:, :],
                    wt_sb[:, c, :],
                    t_T_sb[:, c, n * 512:(n + 1) * 512],
                    start=(c == 0),
                    stop=(c == KC - 1),
                )
            nc.vector.tensor_copy(ht_sb[:, n * 512:(n + 1) * 512], ht_ps[:, :])

        # s_t row [1, R]; r = exp(0.8 s_t) row; bf16 s_t row
        st_row = singles.tile([1, R], F32)
        r_row = singles.tile([1, R], F32)
        st_row_b = singles.tile([1, R], BF16)
        for n in range(R // 512):
            st_ps = misc_psum.tile([1, 512], F32, tag="mps")
            nc.tensor.matmul(
                st_ps[:, :],
                a_t_sb[:, :],
                ht_sb[:, n * 512:(n + 1) * 512],
                start=True,
                stop=True,
            )
            nc.vector.tensor_copy(st_row[:, n * 512:(n + 1) * 512], st_ps[:, :])
            nc.scalar.activation(
                r_row[:, n * 512:(n + 1) * 512], st_ps[:, :], AF.Exp, scale=0.8
            )
        nc.vector.tensor_copy(st_row_b[:, :], st_row[:, :])

        # s_t broadcast to all partitions [128, R] bf16 (via DRAM bounce --
        # partition-step-0 APs are only legal on DRAM sources)
        st_dram = nc.dram_tensor("st_bounce", [1, R], BF16, kind="Internal")
        nc.sync.dma_start(out=st_dram[:, :], in_=st_row_b[0:1, :])
        st_bcast = singles.tile([128, R], BF16)
        nc.sync.dma_start(
            out=st_bcast[:, :],
            in_=bass.AP(tensor=st_dram, offset=0, ap=[[0, 128], [1, R]]),
        )

        # o_input.T resident for projections
        o_T_sb = singles.tile([128, KC, N_O], F32)
        for c in range(KC):
            for h in range(2):
                nc.sync.dma_start(
                    out=o_T_sb[:, c, h * 4096:(h + 1) * 4096],
                    in_=o_T[c * 128:(c + 1) * 128, h * 4096:(h + 1) * 4096],
                )

        # ------------------------------------------------------------------
        # Per-group o-side setup: h_o, s_o, v1/v2, W1ext/W2ext (bf16)
        # ------------------------------------------------------------------
        w1_tiles, w2_tiles, nso_tiles = [], [], []
        for g in range(NG):
            ho_stage = stage.tile([128, GROUP, F_OUT], F32, tag="ho_stage")
            for u in range(0, GROUP, 8):
                ho_ps = misc_psum.tile([128, 8, F_OUT], F32, tag="mps")
                for s in range(8):
                    j0 = (g * GROUP + u + s) * 128
                    for c in range(KC):
                        nc.tensor.matmul(
                            ho_ps[:, s, :],
                            o_T_sb[:, c, j0:j0 + 128],
                            wo_sb[:, c, :],
                            start=(c == 0),
                            stop=(c == KC - 1),
                        )
                nc.vector.tensor_copy(ho_stage[:, u:u + 8, :], ho_ps[:, :, :])

            # s_o[j] = sum_f h_o[j,f] a_o[f]
            so_g = stage.tile([128, GROUP], F32, tag="so")
            prod = stage.tile([128, GROUP, F_OUT], F32, tag="so_prod")
            nc.vector.tensor_tensor(
                prod[:, :, :],
                ho_stage[:, :, :],
                bass.AP(
                    tensor=a_o_b[:, :].tensor,
                    offset=a_o_b[:, :].offset,
                    ap=[a_o_b[:, :].ap[0], [0, GROUP], [1, F_OUT]],
                ),
                OP.mult,
            )
            nc.vector.tensor_reduce(
                so_g[:, :], prod[:, :, :], mybir.AxisListType.X, OP.add
            )
            nso_g = singles.tile([128, GROUP], F32, tag=f"nso{g}")
            nc.vector.tensor_scalar_mul(nso_g[:, :], so_g[:, :], -1.0)
            v1_g = stage.tile([128, GROUP], F32, tag="v1")
            v2_g = stage.tile([128, GROUP], F32, tag="v2")
            nc.scalar.activation(v1_g[:, :], so_g[:, :], AF.Exp)
            nc.scalar.activation(v2_g[:, :], so_g[:, :], AF.Exp, scale=ALPHA)

            w1_g = singles.tile([128, GROUP, F_OUT + 1], BF16, tag=f"w1{g}")
            w2_g = singles.tile([128, GROUP, F_OUT + 1], BF16, tag=f"w2{g}")
            # scaled h_o columns (on gpsimd to offload DVE)
            nc.gpsimd.tensor_tensor(
                w1_g[:, :, 0:F_OUT],
                ho_stage[:, :, :],
                _ap_repeat_free(v1_g[:, :], F_OUT),
                OP.mult,
            )
            nc.gpsimd.tensor_tensor(
                w2_g[:, :, 0:F_OUT],
                ho_stage[:, :, :],
                _ap_repeat_free(v2_g[:, :], F_OUT),
                OP.mult,
            )
            # ones columns carry v1/v2 for the softmax denominator
            nc.vector.tensor_copy(w1_g[:, :, F_OUT], v1_g[:, :])
            nc.vector.tensor_copy(w2_g[:, :, F_OUT], v2_g[:, :])
            w1_tiles.append(w1_g)
            w2_tiles.append(w2_g)
            nso_tiles.append(nso_g)

        # ------------------------------------------------------------------
        # Main loop over j tiles: masks + accumulating matmuls
        # ------------------------------------------------------------------
        NI2 = R // 512  # 2 matmul chunks over i
        t1_acc = [
            acc_psum.tile([F_OUT + 1, 512], F32, tag=f"t1_{n}", name=f"t1_acc{n}")
            for n in range(NI2)
        ]
        t2_acc = [
            acc_psum.tile([F_OUT + 1, 512], F32, tag=f"t2_{n}", name=f"t2_acc{n}")
            for n in range(NI2)
        ]

        for t in range(NJ):
            g, u = divmod(t, GROUP)
            adj_t = adj_pool.tile([128, R], BF16)
            nc.sync.dma_start(out=adj_t[:, :], in_=adjT[t * 128:(t + 1) * 128, :])

            c_t = c_pool.tile([128, R], BF16)
            nc.vector.tensor_scalar(
                c_t[:, :], st_bcast[:, :], nso_tiles[g][:, u:u + 1], None, OP.is_gt
            )
            m1_t = m1_pool.tile([128, R], BF16)
            nc.vector.tensor_tensor(m1_t[:, :], c_t[:, :], adj_t[:, :], OP.mult)
            m2_t = m2_pool.tile([128, R], BF16)
            nc.gpsimd.tensor_tensor(m2_t[:, :], adj_t[:, :], m1_t[:, :], OP.subtract)

            for n in range(NI2):
                nc.tensor.matmul(
                    t1_acc[n][:, :],
                    w1_tiles[g][:, u, :],
                    m1_t[:, n * 512:(n + 1) * 512],
                    start=(t == 0),
                    stop=(t == NJ - 1),
                )
            for n in range(NI2):
                nc.tensor.matmul(
                    t2_acc[n][:, :],
                    w2_tiles[g][:, u, :],
                    m2_t[:, n * 512:(n + 1) * 512],
                    start=(t == 0),
                    stop=(t == NJ - 1),
                )

        # ------------------------------------------------------------------
        # Combine: H = r*T1 + T2 ; out = elu(H[:F] / H[F])
        # ------------------------------------------------------------------
        h_sb = singles.tile([F_OUT + 1, R], F32)
        for n in range(NI2):
            sl = slice(n * 512, (n + 1) * 512)
            rb_ps = misc_psum.tile([F_OUT + 1, 512], F32, tag="mps")
            nc.tensor.matmul(
                rb_ps[:, :], ones_row[:, :], r_row[:, sl], start=True, stop=True
            )
            rb_sb = stage.tile([F_OUT + 1, 512], F32, tag="rb_sb")
            nc.vector.tensor_copy(rb_sb[:, :], rb_ps[:, :])
            nc.vector.tensor_tensor(h_sb[:, sl], rb_sb[:, :], t1_acc[n][:, :], OP.mult)
            nc.vector.tensor_tensor(h_sb[:, sl], h_sb[:, sl], t2_acc[n][:, :], OP.add)

        zr_row = singles.tile([1, R], F32)
        nc.vector.reciprocal(zr_row[:, :], h_sb[F_OUT:F_OUT + 1, :])

        ot_sb = singles.tile([F_OUT, R], F32)
        for n in range(NI2):
            sl = slice(n * 512, (n + 1) * 512)
            zb_ps = misc_psum.tile([F_OUT, 512], F32, tag="mps")
            nc.tensor.matmul(
                zb_ps[:, :], ones_row[:, 0:F_OUT], zr_row[:, sl], start=True, stop=True
            )
            nc.vector.tensor_tensor(ot_sb[:, sl], h_sb[0:F_OUT, sl], zb_ps[:, :], OP.mult)

        # elu(x) = max(x,0) - 1 + exp(min(x,0))
        mn_sb = singles.tile([F_OUT, R], F32)
        ex_sb = singles.tile([F_OUT, R], F32)
        nc.vector.tensor_scalar(mn_sb[:, :], ot_sb[:, :], 0.0, None, OP.min)
        nc.scalar.activation(ex_sb[:, :], mn_sb[:, :], AF.Exp)
        nc.vector.tensor_scalar(ot_sb[:, :], ot_sb[:, :], 0.0, -1.0, OP.max, OP.add)
        nc.vector.tensor_tensor(ot_sb[:, :], ot_sb[:, :], ex_sb[:, :], OP.add)
        nc.sync.dma_start(out=out[:, :], in_=ot_sb[:, :])

    if split_waits:
        _split_multi_waits(nc)
    return nc


_CACHED = {}


def _get_compiled():
    if "nc" not in _CACHED:
        _CACHED["nc"] = build_kernel()
    return _CACHED["nc"]


def kernel(t_input, o_input, W_t, W_o, a, adj, _trace=False):
    from concourse.bass_utils import run_bass_kernel_spmd

    t_input = np.asarray(t_input, dtype=np.float32)
    o_input = np.asarray(o_input, dtype=np.float32)
    W_t = np.asarray(W_t, dtype=np.float32)
    W_o = np.asarray(W_o, dtype=np.float32)
    a = np.asarray(a, dtype=np.float32)
    adj = np.asarray(adj)

    o_T = np.ascontiguousarray(o_input.T)
    adj_b = adj.astype(bf16)

    in_maps = []
    for m in range(N_CORES):
        rows = slice(m * R, (m + 1) * R)
        in_maps.append(
            {
                "t_T": np.ascontiguousarray(t_input[rows, :].T),
                "o_T": o_T,
                "w_t": W_t,
                "w_o": W_o,
                "a_vec": a,
                "adjT": np.ascontiguousarray(adj_b[rows, :].T),
            }
        )

    nc = _get_compiled()
    res = run_bass_kernel_spmd(
        nc, in_maps, core_ids=list(range(N_CORES)), trace=_trace
    )
    out = np.empty((N_T, F_OUT), dtype=np.float32)
    for m in range(N_CORES):
        out[m * R:(m + 1) * R, :] = res.results[m]["out"].T
    if _trace:
        kernel.last_exec_time_ns = res.exec_time_ns
        kernel.last_results = res
    return out



# revision 8
# speedup vs baseline: 1.7588x; 1.7588x over previous
"""Trainium2 Bass kernel for the NodeAttentionLayer (GAT-style) problem.

Math (per reference.py):
    h_t = t_input @ W_t; h_o = o_input @ W_o
    s_t = h_t @ a[:F];  s_o = h_o @ a[F:]
    e[i,j]   = leaky_relu(s_t[i] + s_o[j], 0.2)
    att      = softmax(where(adj>0, e, -9e15), axis=1)
    out      = elu(att @ h_o)

On-device identity: with c = (s_t[i]+s_o[j] > 0), v1 = exp(s_o), v2 =
exp(0.2 s_o), r = exp(0.8 s_t):
    att-numerator @ [h_o|1] = r[i] * (W1 @ M1) + (W2 @ M2)
where W1 = v1*[h_o|1], W2 = v2*[h_o|1], M1 = adj*c, M2 = adj - M1; the
ones column carries the softmax denominator; softmax max-trick cancels.

Engine plan per j-tile [128 j x 1024 i]:
  ACT: c = sigmoid(1e30*(s_t + s_o))       (exact 0/1; ties -> 0.5, benign)
  DVE: M1 = c * adj ; (form A) M2 = adj - M1   (quad-batched [128,4,1024])
  PE : form A: T1 += W1@M1, T2 += W2@M2       (2 streams)
       form B: T1 += W1@M1, T2 += W2@adj + (-W2)@M1  (3 streams, no M2 op)
Form B on a subset of quads balances DVE vs PE. GpSimd is never used for
elementwise work (it shares an SBUF port with DVE and stalls it).

Sharding: rows of t_input/adj (N_t) split across 8 cores; o replicated.
Output computed transposed [F, rows]; host transposes back.
"""

import contextlib
import ctypes
import sys
import types

import ml_dtypes
import numpy as np

import concourse.bass as bass
import concourse.mybir as mybir
import concourse.tile as tile
from concourse.vector_clock import ScopedClock

bf16 = ml_dtypes.bfloat16

# ---------------------------------------------------------------------------
# Environment shims (same as baseline)
# ---------------------------------------------------------------------------

def _patch_tile_drain():
    if getattr(tile.TileContext, "_drain_patch_installed", False):
        return

    def _drain_and_barrier(self, tick_clock, wait_clock):
        nop_inst = self.nc.sync.nop(nofuse=True)
        wait_clock.add_sem_waits(
            nop_inst.ins, ScopedClock({None: tick_clock.global_clock})
        )
        ow = list(nop_inst.ins.sync_info.on_wait) if nop_inst.ins.sync_info else []
        if len(ow) > 1:
            nop_inst.ins.sync_info.on_wait = ow[:1]
            for w in ow[1:]:
                extra = self.nc.sync.nop(nofuse=True)
                if extra.ins.sync_info is None:
                    extra.ins.sync_info = mybir.SyncInfo(on_wait=[w], on_update=[])
                else:
                    extra.ins.sync_info.on_wait = [w]
        self.nc.sync.drain()
        self.nc.all_engine_barrier()
        popped = self.nc._tile_sem_poison_stack.pop()
        assert popped is self._sem_poison
        self.nc.clear_and_free_semaphores(list(self.sems.allocated().values()))
        self.nc.all_engine_barrier()

    tile.TileContext._drain_and_barrier = _drain_and_barrier
    tile.TileContext._drain_patch_installed = True


def _install_ntff_hook():
    if "antenv.axon_hooks" in sys.modules:
        return
    import antenv

    state = {"hook": None}
    mod = types.ModuleType("antenv.axon_hooks")
    mod.set_axon_ntff_profile_hook = lambda h: state.__setitem__("hook", h)
    mod.get_axon_ntff_profile_hook = lambda: state["hook"]
    sys.modules["antenv.axon_hooks"] = mod
    antenv.axon_hooks = mod

    try:
        lib = ctypes.CDLL("/opt/axon/libaxon_pjrt.so")
    except OSError:
        return
    if not hasattr(lib, "axon_start_nrt_profile"):
        return
    lib.axon_start_nrt_profile.argtypes = [
        ctypes.POINTER(ctypes.c_int64),
        ctypes.c_size_t,
    ]
    lib.axon_start_nrt_profile.restype = ctypes.c_int64
    lib.axon_stop_nrt_profile.argtypes = [ctypes.c_char_p]
    lib.axon_stop_nrt_profile.restype = ctypes.c_int64

    @contextlib.contextmanager
    def _ntff_hook(output_dir, device_ids):
        import jax

        jax.devices()
        if device_ids:
            ids = (ctypes.c_int64 * len(device_ids))(*device_ids)
            rc = lib.axon_start_nrt_profile(ids, len(device_ids))
        else:
            rc = lib.axon_start_nrt_profile(None, 0)
        if rc != 0:
            raise RuntimeError(f"axon_start_nrt_profile rc={rc}")
        try:
            yield
        finally:
            n = lib.axon_stop_nrt_profile(str(output_dir).encode())
            print(f"profile: {n} file(s) written to {output_dir}", file=sys.stderr)

    state["hook"] = _ntff_hook


_patch_tile_drain()
_install_ntff_hook()


def _split_multi_waits(nc):
    import bass_rust

    k = 0
    for f in nc.m.functions:
        for blk in f.blocks:
            insts = blk.instructions
            out = []
            changed = False
            for inst in insts:
                si = inst.sync_info
                ow = list(si.on_wait) if si is not None else []
                if len(ow) > 1:
                    for w in ow[:-1]:
                        nop = bass_rust.InstNoOp(
                            name=f"waitsplit-{k}", engine=inst.engine
                        )
                        k += 1
                        nop.sync_info = mybir.SyncInfo(on_wait=[w], on_update=[])
                        out.append(nop)
                    si.on_wait = [ow[-1]]
                    changed = True
                out.append(inst)
            if changed:
                blk.instructions = out


# ---------------------------------------------------------------------------
# Problem constants
# ---------------------------------------------------------------------------
N_T, N_O, F_IN, F_OUT = 8192, 8192, 256, 64
N_CORES = 8
R = N_T // N_CORES            # 1024 t-rows per core
NJ = N_O // 128               # 64 j-tiles
NQ = NJ // 4                  # 16 quads (adj DMA batches of 4 tiles)
KC = F_IN // 128              # 2 contraction chunks
F32 = mybir.dt.float32
BF16 = mybir.dt.bfloat16
AF = mybir.ActivationFunctionType
OP = mybir.AluOpType

# Quads processed in "form B" (3 PE streams, no M2 on DVE)
FORM_B_QUADS = frozenset({1, 3, 6, 9, 11, 14})


def _rep_free(ap, reps):
    """View [P, K] tile as [P, K, reps] via innermost step-0."""
    return bass.AP(tensor=ap.tensor, offset=ap.offset, ap=list(ap.ap) + [[0, reps]])


def build_kernel(split_waits=True):
    nc = bass.Bass("TRN2")

    t_T = nc.dram_tensor("t_T", [F_IN, R], BF16, kind="ExternalInput")
    o_T = nc.dram_tensor("o_T", [F_IN, N_O], BF16, kind="ExternalInput")
    wt_d = nc.dram_tensor("wt_d", [F_IN, F_OUT], BF16, kind="ExternalInput")
    wo_d = nc.dram_tensor("wo_d", [F_IN, F_OUT], BF16, kind="ExternalInput")
    a_d = nc.dram_tensor("a_d", [2 * F_OUT, 1], BF16, kind="ExternalInput")
    adjT = nc.dram_tensor("adjT", [N_O, R], BF16, kind="ExternalInput")
    out_d = nc.dram_tensor("out_d", [F_OUT, R], F32, kind="ExternalOutput")

    st_dram = nc.dram_tensor("st_dram", [1, R], BF16, kind="Internal")
    hoT_dram = nc.dram_tensor("hoT_dram", [F_OUT, N_O], BF16, kind="Internal")
    so_dram = nc.dram_tensor("so_dram", [1, N_O], F32, kind="Internal")

    with tile.TileContext(nc) as tc, contextlib.ExitStack() as ctx:
        S = ctx.enter_context(tc.tile_pool(name="singles", bufs=1))
        adj_pool = ctx.enter_context(tc.tile_pool(name="adj", bufs=3))
        c_pool = ctx.enter_context(tc.tile_pool(name="cq", bufs=2))
        m1_pool = ctx.enter_context(tc.tile_pool(name="m1q", bufs=3))
        m2_pool = ctx.enter_context(tc.tile_pool(name="m2q", bufs=2))
        acc = ctx.enter_context(tc.tile_pool(name="acc", bufs=1, space="PSUM"))
        mps = ctx.enter_context(tc.tile_pool(name="mps", bufs=4, space="PSUM"))

        # ------------------------------------------------------------------
        # Head: weights + t-side
        # ------------------------------------------------------------------
        wt_sb = S.tile([128, KC, F_OUT], BF16)
        wo_sb = S.tile([128, KC, F_OUT], BF16)
        for c in range(KC):
            nc.sync.dma_start(out=wt_sb[:, c, :], in_=wt_d[c * 128:(c + 1) * 128, :])
            nc.sync.dma_start(out=wo_sb[:, c, :], in_=wo_d[c * 128:(c + 1) * 128, :])
        a_t = S.tile([F_OUT, 1], BF16)
        nc.sync.dma_start(out=a_t[:, :], in_=a_d[0:F_OUT, :])
        a_o = S.tile([F_OUT, 1], BF16)
        nc.sync.dma_start(out=a_o[:, :], in_=a_d[F_OUT:2 * F_OUT, :])
        t_sb = S.tile([128, KC, R], BF16)
        for c in range(KC):
            nc.sync.dma_start(out=t_sb[:, c, :], in_=t_T[c * 128:(c + 1) * 128, :])
        # o_T staged in two half-waves (halves SBUF footprint)
        o_pool = ctx.enter_context(tc.tile_pool(name="op", bufs=2))

        # PE warm-up (junk matmuls on weight tiles; heats HAM window)
        warm_ps = mps.tile([F_OUT, 512], F32, tag="mps")
        for i in range(10):
            nc.tensor.matmul(warm_ps[:, :], wt_sb[:, 0, :],
                             t_sb[:, 0, 0:512], start=True, stop=True)

        # h_tT [64, R] = W_t.T-chunks vs t chunks; then s_t row, r row
        ht_sb = S.tile([F_OUT, R], BF16)
        for n in range(R // 512):
            ht_ps = mps.tile([F_OUT, 512], F32, tag="mps")
            for c in range(KC):
                nc.tensor.matmul(ht_ps[:, :], wt_sb[:, c, :],
                                 t_sb[:, c, n * 512:(n + 1) * 512],
                                 start=(c == 0), stop=(c == KC - 1))
            nc.vector.tensor_copy(ht_sb[:, n * 512:(n + 1) * 512], ht_ps[:, :])

        st_b = S.tile([1, R], BF16)
        r_b = S.tile([1, R], BF16)
        for n in range(R // 512):
            st_ps = mps.tile([1, 512], F32, tag="mps")
            nc.tensor.matmul(st_ps[:, :], a_t[:, :],
                             ht_sb[:, n * 512:(n + 1) * 512], start=True, stop=True)
            nc.vector.tensor_copy(st_b[:, n * 512:(n + 1) * 512], st_ps[:, :])
            nc.scalar.activation(r_b[:, n * 512:(n + 1) * 512], st_ps[:, :],
                                 AF.Exp, scale=0.8)

        # st broadcast to 128 partitions via DRAM bounce
        nc.sync.dma_start(out=st_dram[:, :], in_=st_b[0:1, :])
        st_bcast = S.tile([128, R], BF16)
        nc.sync.dma_start(
            out=st_bcast[:, :],
            in_=bass.AP(tensor=st_dram, offset=0, ap=[[0, 128], [1, R]]),
        )

        # ------------------------------------------------------------------
        # Head: h_o transposed + s_o + transpose back
        # ------------------------------------------------------------------
        hoT_pool = ctx.enter_context(tc.tile_pool(name="hp", bufs=1))
        so_row = S.tile([1, N_O], F32)
        for w in range(2):
            hoT_h = hoT_pool.tile([F_OUT, N_O // 2], BF16, tag="hoth",
                                  name=f"hoth{w}")
            for ww in range(2):
                o_half = o_pool.tile([128, KC, 2048], BF16, tag="oh",
                                     name=f"oh{w}_{ww}")
                for c in range(KC):
                    nc.scalar.dma_start(
                        out=o_half[:, c, :],
                        in_=o_T[c * 128:(c + 1) * 128,
                                (2 * w + ww) * 2048:(2 * w + ww + 1) * 2048],
                    )
                for hh in range(4):
                    h = ww * 4 + hh
                    hoT_ps = mps.tile([F_OUT, 512], F32, tag="mps")
                    for c in range(KC):
                        nc.tensor.matmul(hoT_ps[:, :], wo_sb[:, c, :],
                                         o_half[:, c, hh * 512:(hh + 1) * 512],
                                         start=(c == 0), stop=(c == KC - 1))
                    nc.vector.tensor_copy(hoT_h[:, h * 512:(h + 1) * 512],
                                          hoT_ps[:, :])
            for h in range(8):
                so_ps = mps.tile([1, 512], F32, tag="mps")
                nc.tensor.matmul(so_ps[:, :], a_o[:, :],
                                 hoT_h[:, h * 512:(h + 1) * 512],
                                 start=True, stop=True)
                nc.vector.tensor_copy(
                    so_row[:, (w * 8 + h) * 512:(w * 8 + h + 1) * 512],
                    so_ps[:, :])
            nc.scalar.dma_start(out=hoT_dram[:, w * 4096:(w + 1) * 4096],
                                in_=hoT_h[:, :])

        ho_sb = S.tile([128, NJ, F_OUT], BF16)
        nc.scalar.dma_start_transpose(ho_sb[:, :, :], hoT_dram[:, :])

        # bounce so -> DRAM -> [128, NJ] layout
        nc.sync.dma_start(out=so_dram[:, :], in_=so_row[0:1, :])
        so_sb = S.tile([128, NJ], F32)
        nc.sync.dma_start(
            out=so_sb[:, :],
            in_=bass.AP(tensor=so_dram, offset=0, ap=[[1, 128], [128, NJ]]),
        )

        so30 = S.tile([128, NJ], F32)
        nc.vector.tensor_scalar(so30[:, :], so_sb[:, :], 1.0e30, None, OP.mult)
        v1_b = S.tile([128, NJ], BF16)
        v2_b = S.tile([128, NJ], BF16)
        nc.scalar.activation(v1_b[:, :], so_sb[:, :], AF.Exp)
        nc.scalar.activation(v2_b[:, :], so_sb[:, :], AF.Exp, scale=0.2)

        # trigger sigmoid table load early (all exps are done above)
        sig_dummy = S.tile([1, NJ], BF16)
        nc.scalar.activation(sig_dummy[:, :], so_sb[0:1, :], AF.Sigmoid,
                             scale=1.0e30)

        # ------------------------------------------------------------------
        # Head: stationary weights W1 = v1*[h_o|1], W2 = v2*[h_o|1], W2n=-W2
        # ------------------------------------------------------------------
        w1_all = S.tile([128, NJ, F_OUT + 1], BF16)
        w2_all = S.tile([128, NJ, F_OUT + 1], BF16)
        w2n_all = S.tile([128, NJ, F_OUT + 1], BF16)
        G = 16  # tiles per build group
        for g in range(NJ // G):
            sl = slice(g * G, (g + 1) * G)
            nc.vector.tensor_tensor(
                w1_all[:, sl, 0:F_OUT], ho_sb[:, sl, :],
                _rep_free(v1_b[:, sl], F_OUT), OP.mult)
            nc.vector.tensor_copy(w1_all[:, sl, F_OUT], v1_b[:, sl])
            nc.vector.tensor_tensor(
                w2_all[:, sl, 0:F_OUT], ho_sb[:, sl, :],
                _rep_free(v2_b[:, sl], F_OUT), OP.mult)
            nc.vector.tensor_copy(w2_all[:, sl, F_OUT], v2_b[:, sl])
            nc.vector.tensor_scalar(w2n_all[:, sl, :], w2_all[:, sl, :],
                                    -1.0, None, OP.mult)

        # ------------------------------------------------------------------
        # Main loop over quads of 4 j-tiles
        # ------------------------------------------------------------------
        t1_acc = [acc.tile([F_OUT + 1, 512], F32, tag=f"t1_{n}", name=f"t1_{n}")
                  for n in range(2)]
        t2_acc = [acc.tile([F_OUT + 1, 512], F32, tag=f"t2_{n}", name=f"t2_{n}")
                  for n in range(2)]
        t1_started = [False, False]
        t2_started = [False, False]

        for q in range(NQ):
            batch = adj_pool.tile([128, 4, R], BF16, tag="adj", name=f"adj{q}")
            nc.sync.dma_start(
                out=batch[:, :, :],
                in_=bass.AP(tensor=adjT, offset=q * 512 * R,
                            ap=[[R, 128], [128 * R, 4], [1, R]]),
            )
            cq = c_pool.tile([128, 4, R], BF16, tag="cq", name=f"cq{q}")
            for s in range(4):
                t = q * 4 + s
                nc.scalar.activation(cq[:, s, :], st_bcast[:, :], AF.Sigmoid,
                                     bias=so30[:, t:t + 1], scale=1.0e30)
            m1q = m1_pool.tile([128, 4, R], BF16, tag="m1q", name=f"m1q{q}")
            nc.vector.tensor_tensor(m1q[:, :, :], cq[:, :, :], batch[:, :, :],
                                    OP.mult)
            form_b = q in FORM_B_QUADS
            if not form_b:
                m2q = m2_pool.tile([128, 4, R], BF16, tag="m2q", name=f"m2q{q}")
                nc.vector.tensor_tensor(m2q[:, :, :], batch[:, :, :],
                                        m1q[:, :, :], OP.subtract)
            last_q = q == NQ - 1
            for s in range(4):
                t = q * 4 + s
                last_t = last_q and s == 3
                for n in range(2):
                    sl = slice(n * 512, (n + 1) * 512)
                    nc.tensor.matmul(t1_acc[n][:, :], w1_all[:, t, :],
                                     m1q[:, s, sl],
                                     start=not t1_started[n], stop=last_t)
                    t1_started[n] = True
                if form_b:
                    for n in range(2):
                        sl = slice(n * 512, (n + 1) * 512)
                        nc.tensor.matmul(t2_acc[n][:, :], w2_all[:, t, :],
                                         batch[:, s, sl],
                                         start=not t2_started[n], stop=False)
                        t2_started[n] = True
                    for n in range(2):
                        sl = slice(n * 512, (n + 1) * 512)
                        nc.tensor.matmul(t2_acc[n][:, :], w2n_all[:, t, :],
                                         m1q[:, s, sl],
                                         start=False, stop=last_t)
                else:
                    for n in range(2):
                        sl = slice(n * 512, (n + 1) * 512)
                        nc.tensor.matmul(t2_acc[n][:, :], w2_all[:, t, :],
                                         m2q[:, s, sl],
                                         start=not t2_started[n], stop=last_t)
                        t2_started[n] = True

        # ------------------------------------------------------------------
        # Tail: H = r*T1 + T2; out = elu(H[:F]/H[F])
        # ------------------------------------------------------------------
        ones65 = S.tile([1, F_OUT + 1], BF16)
        nc.vector.memset(ones65[:, :], 1.0)

        h_sb = S.tile([F_OUT + 1, R], F32)
        for n in range(2):
            sl = slice(n * 512, (n + 1) * 512)
            rb_ps = mps.tile([F_OUT + 1, 512], F32, tag="mps")
            nc.tensor.matmul(rb_ps[:, :], ones65[:, :], r_b[:, sl],
                             start=True, stop=True)
            nc.vector.tensor_copy(h_sb[:, sl], rb_ps[:, :])
            nc.vector.tensor_tensor(h_sb[:, sl], h_sb[:, sl], t1_acc[n][:, :],
                                    OP.mult)
            nc.vector.tensor_tensor(h_sb[:, sl], h_sb[:, sl], t2_acc[n][:, :],
                                    OP.add)

        zr = S.tile([1, R], F32)
        nc.vector.reciprocal(zr[:, :], h_sb[F_OUT:F_OUT + 1, :])
        zr_b = S.tile([1, R], BF16)
        nc.vector.tensor_copy(zr_b[:, :], zr[:, :])

        ot_sb = S.tile([F_OUT, R], F32)
        for n in range(2):
            sl = slice(n * 512, (n + 1) * 512)
            zb_ps = mps.tile([F_OUT, 512], F32, tag="mps")
            nc.tensor.matmul(zb_ps[:, :], ones65[:, 0:F_OUT], zr_b[:, sl],
                             start=True, stop=True)
            nc.vector.tensor_tensor(ot_sb[:, sl], h_sb[0:F_OUT, sl], zb_ps[:, :],
                                    OP.mult)

        # elu(x) = max(x,0) - 1 + exp(min(x,0))
        mn_sb = S.tile([F_OUT, R], F32)
        nc.vector.tensor_scalar(mn_sb[:, :], ot_sb[:, :], 0.0, None, OP.min)
        nc.scalar.activation(mn_sb[:, :], mn_sb[:, :], AF.Exp)
        nc.vector.tensor_scalar(ot_sb[:, :], ot_sb[:, :], 0.0, -1.0, OP.max, OP.add)
        nc.vector.tensor_tensor(ot_sb[:, :], ot_sb[:, :], mn_sb[:, :], OP.add)
        nc.sync.dma_start(out=out_d[:, :], in_=ot_sb[:, :])

    if split_waits:
        _split_multi_waits(nc)
    return nc


_CACHED = {}


def _get_compiled():
    if "nc" not in _CACHED:
        _CACHED["nc"] = build_kernel()
    return _CACHED["nc"]


def kernel(t_input, o_input, W_t, W_o, a, adj, _trace=False):
    from concourse.bass_utils import run_bass_kernel_spmd

    t_input = np.asarray(t_input, dtype=np.float32)
    o_input = np.asarray(o_input, dtype=np.float32)
    W_t = np.asarray(W_t, dtype=np.float32)
    W_o = np.asarray(W_o, dtype=np.float32)
    a = np.asarray(a, dtype=np.float32)
    adj = np.asarray(adj)

    o_T = np.ascontiguousarray(o_input.T).astype(bf16)
    wt_b = W_t.astype(bf16)
    wo_b = W_o.astype(bf16)
    a_b = a.astype(bf16)
    adj_b = adj.astype(bf16)

    in_maps = []
    for m in range(N_CORES):
        rows = slice(m * R, (m + 1) * R)
        in_maps.append(
            {
                "t_T": np.ascontiguousarray(t_input[rows, :].T).astype(bf16),
                "o_T": o_T,
                "wt_d": wt_b,
                "wo_d": wo_b,
                "a_d": a_b,
                "adjT": np.ascontiguousarray(adj_b[rows, :].T),
            }
        )

    nc = _get_compiled()
    res = run_bass_kernel_spmd(
        nc, in_maps, core_ids=list(range(N_CORES)), trace=_trace
    )
    out = np.empty((N_T, F_OUT), dtype=np.float32)
    for m in range(N_CORES):
        out[m * R:(m + 1) * R, :] = res.results[m]["out_d"].T
    if _trace:
        kernel.last_exec_time_ns = res.exec_time_ns
        kernel.last_results = res
    return out
